# revision 6
# baseline (speedup 1.0000x reference)
"""Chamfer distance (bidirectional NN min-squared-distance) on 8 Trainium2 cores.

Strategy
--------
reference computes, per batch b (4 batches):
    dist1[b, i] = min_j ||xyz1[b,i] - xyz2[b,j]||^2      (16384 queries vs 16384 refs)
    dist2[b, j] = min_i ||xyz2[b,j] - xyz1[b,i]||^2
That is 8 independent "NN-min" jobs (4 batches x 2 directions) -> one job per
NeuronCore (SPMD, identical program, different data).

Per job, the host splits the queries into 128 spatially tight blocks of 128
points via a balanced KD partition (recursive median splits). For each block it
gathers the WE=256 reference points nearest to the block's bounding box (by
point-to-box distance) into a dense panel. The device computes, per block, the
squared distance of its 128 queries to the 256 panel refs as one bf16 matmul
and takes a free-dim min-reduce on the vector engine.

The squared distance is expressed as a K=30 bf16 matmul: each point is lifted
to 5 features
  queries:  (x, y, z, |q|^2, 1)        refs: (-2x, -2y, -2z, 1, |r|^2)
so d(i,j) = fa_i . gb_j, and every feature is split into three bf16 limbs
(hi/mid/lo) with the 6 significant cross-products kept, giving ~1e-5 absolute
accuracy (fp32-class) at full bf16 PE rate.

Exactness is restored on the host: every ref excluded from a block's panel is
at box-distance > thr from the block's bbox, so for a query q inside the bbox
the excluded refs are all farther than thr + dist(q, bbox boundary). A query's
panel-min is provably the global min when sqrt(md + E_DEV) <= thr + d_q. The
few queries that fail this certificate (~2-8%) are recomputed exactly on the
host against all refs.
"""

import numpy as np
import ml_dtypes

import concourse.bass as bass
import concourse.mybir as mybir
import concourse.tile as tile
from concourse import bacc
from concourse.bass_utils import run_bass_kernel_spmd

N = 16384
P = 128                  # partition block of queries
NBLK = N // P            # 128 query blocks
WE = 224                 # panel width (refs per block)
PSW = 256                # PSUM slot width per block (1KB-aligned half banks)
KF = 5                   # base features
K = 6 * KF               # bf16 triple-split term groups
E_DEV = 3.0e-5           # conservative device abs-error bound used by the certificate
GCHUNK = 8               # query blocks per gT DMA chunk (double-buffered)
RGRP = 8                 # query blocks per vector min-reduce (4 PSUM banks)

_CACHE = {}

# ----------------------------------------------------------------- device ---

def _build_nc():
    nc = bacc.Bacc("TRN2", target_bir_lowering=False, debug=False)
    aT = nc.dram_tensor("aT", [K, N], mybir.dt.bfloat16, kind="ExternalInput").ap()
    gT = nc.dram_tensor("gT", [K, NBLK * WE], mybir.dt.bfloat16, kind="ExternalInput").ap()
    md = nc.dram_tensor("md", [P, NBLK], mybir.dt.float32, kind="ExternalOutput").ap()
    with tile.TileContext(nc) as tc:
        with tc.tile_pool(name="abig", bufs=1) as abig, \
             tc.tile_pool(name="gch", bufs=2) as gch, \
             tc.tile_pool(name="strip", bufs=1) as stp, \
             tc.tile_pool(name="ps", bufs=2, space="PSUM") as psp:
            a_sb = abig.tile([K, N], mybir.dt.bfloat16, tag="a")
            # query DMA on the sync-engine queue so it doesn't delay the
            # gpsimd-issued panel chunks; split so block 0's weights land early
            AC = 2
            for i in range(AC):
                s = (N // AC) * i
                nc.sync.dma_start(a_sb[:, s:s + N // AC], aT[:, s:s + N // AC])
            strip = stp.tile([P, NBLK], mybir.dt.float32)
            for c in range(NBLK // GCHUNK):
                g_sb = gch.tile([K, GCHUNK * WE], mybir.dt.bfloat16, tag="g")
                nc.gpsimd.dma_start(
                    g_sb[:], gT[:, c * GCHUNK * WE:(c + 1) * GCHUNK * WE])
                for rg in range(GCHUNK // RGRP):
                    # PSW-wide PSUM slots keep each matmul output inside
                    # 1KB-aligned half banks; only [:WE] is written/reduced
                    ps = psp.tile([P, RGRP, PSW], mybir.dt.float32)
                    for j in range(RGRP):
                        t = c * GCHUNK + rg * RGRP + j
                        w0 = (rg * RGRP + j) * WE
                        nc.tensor.matmul(
                            ps[:, j, 0:WE],
                            lhsT=a_sb[:, P * t:P * (t + 1)],
                            rhs=g_sb[:, w0:w0 + WE],
                            start=True, stop=True,
                        )
                    t0 = c * GCHUNK + rg * RGRP
                    nc.vector.tensor_reduce(
                        out=strip[:, t0:t0 + RGRP], in_=ps[:, :, 0:WE],
                        axis=mybir.AxisListType.X, op=mybir.AluOpType.min,
                    )
            nc.gpsimd.dma_start(md[:, :], strip[:])
    nc.finalize()
    return nc


def _get_nc():
    if "nc" not in _CACHE:
        _CACHE["nc"] = _build_nc()
    return _CACHE["nc"]

# ------------------------------------------------------------------- host ---

def _split3(f32):
    """fp32 array -> 3 bf16 limbs (hi, mid, lo), f ~= h + m + l."""
    h = f32.astype(ml_dtypes.bfloat16)
    r = f32 - h.astype(np.float32)
    m = r.astype(ml_dtypes.bfloat16)
    l = (r - m.astype(np.float32)).astype(ml_dtypes.bfloat16)
    return h, m, l


def _query_feats(p):
    n2 = (p * p).sum(1, keepdims=True)
    one = np.ones((len(p), 1), np.float32)
    return np.concatenate([p, n2, one], 1).astype(np.float32)       # [n, 5]


def _ref_feats(p):
    n2 = (p * p).sum(1, keepdims=True)
    one = np.ones((len(p), 1), np.float32)
    return np.concatenate([-2.0 * p, one, n2], 1).astype(np.float32)  # [n, 5]


def _lift(fa, gb):
    """[n,5] fp32 pairs -> K=30 bf16 rows so that aT.T @ gT ~= fa @ gb.T."""
    ah, am, al = _split3(fa)
    bh, bm, bl = _split3(gb)
    aT = np.concatenate([ah, ah, ah, am, am, al], 1).T.copy()  # [30, n]
    gT = np.concatenate([bh, bm, bl, bh, bm, bh], 1).T.copy()  # [30, n]
    return aT, gT


def _kd_partition(q):
    """Balanced KD partition: permutation so consecutive P-point groups are
    spatially tight (recursive median split along the widest dimension)."""
    groups = [np.arange(N)]
    while len(groups) < NBLK:
        new = []
        for g in groups:
            pts = q[g]
            dim = int(np.argmax(pts.max(0) - pts.min(0)))
            half = len(g) // 2
            part = np.argpartition(pts[:, dim], half)
            new.append(g[part[:half]])
            new.append(g[part[half:]])
        groups = new
    return np.concatenate(groups)


def _build_panels(qs, r):
    """qs [N,3] block-ordered queries; r [N,3] refs.
    Returns panel ref indices [NBLK, WE], certificate radius thr [NBLK] (fp64,
    safe lower bound on the distance from any excluded ref to the block bbox),
    and per-query distance to own-bbox boundary dq [NBLK, P] (fp64)."""
    qb = qs.reshape(NBLK, P, 3).astype(np.float64)
    lo = qb.min(1)
    hi = qb.max(1)
    r64 = r.astype(np.float64)
    d = np.maximum(lo[:, None, :] - r64[None, :, :], 0.0) \
        + np.maximum(r64[None, :, :] - hi[:, None, :], 0.0)
    bd2 = np.einsum('bnk,bnk->bn', d, d)                  # [NBLK, N]
    idx = np.argpartition(bd2, WE - 1, axis=1)[:, :WE]
    thr = np.sqrt(np.partition(bd2, WE - 1, axis=1)[:, WE - 1])
    dq = np.minimum(qb - lo[:, None, :], hi[:, None, :] - qb).min(-1)  # [NBLK, P]
    return idx, thr, dq


def _exact_rows(q, r, rows):
    """Exact min squared distance (fp64) for query rows `rows` against all refs."""
    out = np.empty(len(rows))
    r64 = r.astype(np.float64)
    CH = 2048
    for s in range(0, len(rows), CH):
        qq = q[rows[s:s + CH]].astype(np.float64)
        d = ((qq[:, None, :] - r64[None, :, :]) ** 2).sum(-1)
        out[s:s + CH] = d.min(1)
    return out


def _finish_job(md_strip, qs, rs, thr, dq):
    """md_strip [P, NBLK] device panel-mins for block-ordered queries; verify +
    repair. Returns md for block-ordered queries [N] (float64)."""
    md = np.maximum(md_strip.T.reshape(N).astype(np.float64), 0.0)
    bound = (thr[:, None] + dq).reshape(N)                # sound exclusion radius
    bad = np.flatnonzero(np.sqrt(md + E_DEV) > bound)
    _CACHE.setdefault("repairs", []).append(len(bad))
    if len(bad):
        md[bad] = _exact_rows(qs, rs, bad)
    return md


def kernel(xyz1: np.ndarray, xyz2: np.ndarray):
    xyz1 = np.asarray(xyz1, dtype=np.float32)
    xyz2 = np.asarray(xyz2, dtype=np.float32)
    B = xyz1.shape[0]
    assert xyz1.shape == (B, N, 3) and xyz2.shape == (B, N, 3)

    # 8 jobs: (batch, direction). direction 0: queries=xyz1 refs=xyz2 -> dist1
    jobs = []
    for b in range(B):
        jobs.append((xyz1[b], xyz2[b]))
        jobs.append((xyz2[b], xyz1[b]))

    in_maps = []
    host_state = []
    for (q, r) in jobs:
        oq = _kd_partition(q)
        qs = q[oq]
        idx, thr, dq = _build_panels(qs, r)
        panel = r[idx.reshape(-1)]                         # [NBLK*WE, 3]
        aT, _ = _lift(_query_feats(qs), _ref_feats(qs[:1]))
        _, gT = _lift(_query_feats(panel[:1]), _ref_feats(panel))
        in_maps.append({"aT": aT, "gT": gT})
        host_state.append((qs, r, oq, thr, dq))

    nc = _get_nc()
    _CACHE["last_in_maps"] = in_maps
    res = run_bass_kernel_spmd(nc, in_maps, core_ids=list(range(len(jobs))))
    _CACHE["last_results"] = res

    dist1 = np.empty((B, N), np.float32)
    dist2 = np.empty((B, N), np.float32)
    for j, (qs, r, oq, thr, dq) in enumerate(host_state):
        md_sorted = _finish_job(res.results[j]["md"], qs, r, thr, dq)
        md = np.empty(N, np.float64)
        md[oq] = md_sorted
        if j % 2 == 0:
            dist1[j // 2] = md.astype(np.float32)
        else:
            dist2[j // 2] = md.astype(np.float32)
    return dist1, dist2


# revision 8
# speedup vs baseline: 1.9244x; 1.9244x over previous
"""Chamfer distance (bidirectional NN min-squared-distance) on 8 Trainium2 cores.

Strategy
--------
reference computes, per batch b (4 batches):
    dist1[b, i] = min_j ||xyz1[b,i] - xyz2[b,j]||^2      (16384 queries vs 16384 refs)
    dist2[b, j] = min_i ||xyz2[b,j] - xyz1[b,i]||^2
That is 8 independent "NN-min" jobs (4 batches x 2 directions) -> one job per
NeuronCore (SPMD, identical program, different data).

Per job, the host splits the queries into 128 spatially tight blocks of 128
points via a balanced KD partition (recursive median splits). For each block it
gathers the WE=224 reference points nearest to the block's bounding box (by
point-to-box distance) into a dense panel. The device computes, per block, the
squared distance of its 128 queries to the 224 panel refs as one bf16 matmul
and takes a free-dim min-reduce on the vector engine.

The squared distance is expressed as a K=30 bf16 matmul: each point is lifted
to 5 features
  queries:  (x, y, z, |q|^2, 1)        refs: (-2x, -2y, -2z, 1, |r|^2)
so d(i,j) = fa_i . gb_j, and every feature is split into three bf16 limbs
(hi/mid/lo) with the 6 significant cross-products kept, giving ~1e-5 absolute
accuracy (fp32-class) at full bf16 PE rate.

Exactness is restored on the host: every ref excluded from a block's panel is
at box-distance > thr from the block's bbox, so for a query q inside the bbox
the excluded refs are all farther than thr + dist(q, bbox boundary). A query's
panel-min is provably the global min when sqrt(md + E_DEV) <= thr + d_q. The
few queries that fail this certificate (~13-14%) are recomputed exactly on the
host against all refs.
"""

import numpy as np
import ml_dtypes

import concourse.bass as bass
import concourse.mybir as mybir
import concourse.tile as tile
from concourse import bacc
from concourse.bass_utils import run_bass_kernel_spmd

N = 16384
P = 128                  # partition block of queries
NBLK = N // P            # 128 query blocks
WE = 224                 # panel width (refs per block)
PSW = 256                # PSUM slot width per block (1KB-aligned half banks)
KF = 5                   # base features
K = 6 * KF               # bf16 triple-split term groups
E_DEV = 3.0e-5           # conservative device abs-error bound used by the certificate
GCHUNK = 8               # query blocks per gT DMA chunk (double-buffered)
RGRP = 8                 # query blocks per vector min-reduce (4 PSUM banks)

_CACHE = {}

# ----------------------------------------------------------------- device ---

def _build_nc():
    nc = bacc.Bacc("TRN2", target_bir_lowering=False, debug=False)
    aT = nc.dram_tensor("aT", [K, N], mybir.dt.bfloat16, kind="ExternalInput").ap()
    gT = nc.dram_tensor("gT", [K, NBLK * WE], mybir.dt.bfloat16, kind="ExternalInput").ap()
    md = nc.dram_tensor("md", [P, NBLK], mybir.dt.float32, kind="ExternalOutput").ap()
    with tile.TileContext(nc) as tc:
        with tc.tile_pool(name="abig", bufs=1) as abig, \
             tc.tile_pool(name="gch", bufs=2) as gch, \
             tc.tile_pool(name="strip", bufs=1) as stp, \
             tc.tile_pool(name="ps", bufs=2, space="PSUM") as psp:
            a_sb = abig.tile([K, N], mybir.dt.bfloat16, tag="a")
            # query DMA on the sync-engine queue so it doesn't delay the
            # gpsimd-issued panel chunks; split so block 0's weights land early
            AC = 2
            for i in range(AC):
                s = (N // AC) * i
                nc.sync.dma_start(a_sb[:, s:s + N // AC], aT[:, s:s + N // AC])
            strip = stp.tile([P, NBLK], mybir.dt.float32)
            for c in range(NBLK // GCHUNK):
                g_sb = gch.tile([K, GCHUNK * WE], mybir.dt.bfloat16, tag="g")
                nc.gpsimd.dma_start(
                    g_sb[:], gT[:, c * GCHUNK * WE:(c + 1) * GCHUNK * WE])
                for rg in range(GCHUNK // RGRP):
                    # PSW-wide PSUM slots keep each matmul output inside
                    # 1KB-aligned half banks; only [:WE] is written/reduced
                    ps = psp.tile([P, RGRP, PSW], mybir.dt.float32)
                    for j in range(RGRP):
                        t = c * GCHUNK + rg * RGRP + j
                        w0 = (rg * RGRP + j) * WE
                        nc.tensor.matmul(
                            ps[:, j, 0:WE],
                            lhsT=a_sb[:, P * t:P * (t + 1)],
                            rhs=g_sb[:, w0:w0 + WE],
                            start=True, stop=True,
                        )
                    t0 = c * GCHUNK + rg * RGRP
                    nc.vector.tensor_reduce(
                        out=strip[:, t0:t0 + RGRP], in_=ps[:, :, 0:WE],
                        axis=mybir.AxisListType.X, op=mybir.AluOpType.min,
                    )
            nc.gpsimd.dma_start(md[:, :], strip[:])
    nc.finalize()
    return nc


def _get_nc():
    if "nc" not in _CACHE:
        _CACHE["nc"] = _build_nc()
    return _CACHE["nc"]

# ------------------------------------------------------------------- host ---

def _split3(f32):
    """fp32 array -> 3 bf16 limbs (hi, mid, lo), f ~= h + m + l."""
    h = f32.astype(ml_dtypes.bfloat16)
    r = f32 - h.astype(np.float32)
    m = r.astype(ml_dtypes.bfloat16)
    l = (r - m.astype(np.float32)).astype(ml_dtypes.bfloat16)
    return h, m, l


def _query_feats(p):
    n2 = (p * p).sum(1, keepdims=True)
    one = np.ones((len(p), 1), np.float32)
    return np.concatenate([p, n2, one], 1).astype(np.float32)       # [n, 5]


def _ref_feats(p):
    n2 = (p * p).sum(1, keepdims=True)
    one = np.ones((len(p), 1), np.float32)
    return np.concatenate([-2.0 * p, one, n2], 1).astype(np.float32)  # [n, 5]


def _lift(fa, gb):
    """[n,5] fp32 pairs -> K=30 bf16 rows so that aT.T @ gT ~= fa @ gb.T."""
    ah, am, al = _split3(fa)
    bh, bm, bl = _split3(gb)
    aT = np.concatenate([ah, ah, ah, am, am, al], 1).T.copy()  # [30, n]
    gT = np.concatenate([bh, bm, bl, bh, bm, bh], 1).T.copy()  # [30, n]
    return aT, gT


def _kd_partition(q):
    """Balanced KD partition: permutation so consecutive P-point groups are
    spatially tight (recursive median split along the widest dimension)."""
    groups = [np.arange(N)]
    while len(groups) < NBLK:
        new = []
        for g in groups:
            pts = q[g]
            dim = int(np.argmax(pts.max(0) - pts.min(0)))
            half = len(g) // 2
            part = np.argpartition(pts[:, dim], half)
            new.append(g[part[:half]])
            new.append(g[part[half:]])
        groups = new
    return np.concatenate(groups)


def _build_panels(qs, r):
    """qs [N,3] block-ordered queries; r [N,3] refs.
    Returns panel ref indices [NBLK, WE], certificate radius thr [NBLK] (fp64,
    safe lower bound on the distance from any excluded ref to the block bbox),
    and per-query distance to own-bbox boundary dq [NBLK, P] (fp64)."""
    qb = qs.reshape(NBLK, P, 3).astype(np.float64)
    lo = qb.min(1)
    hi = qb.max(1)
    r64 = r.astype(np.float64)
    d = np.maximum(lo[:, None, :] - r64[None, :, :], 0.0) \
        + np.maximum(r64[None, :, :] - hi[:, None, :], 0.0)
    bd2 = np.einsum('bnk,bnk->bn', d, d)                  # [NBLK, N]
    idx = np.argpartition(bd2, WE - 1, axis=1)[:, :WE]
    thr = np.sqrt(np.partition(bd2, WE - 1, axis=1)[:, WE - 1])
    dq = np.minimum(qb - lo[:, None, :], hi[:, None, :] - qb).min(-1)  # [NBLK, P]
    return idx, thr, dq


def _exact_rows(q, r, rows):
    """Exact min squared distance (fp64) for query rows `rows` against all refs."""
    out = np.empty(len(rows))
    r64 = r.astype(np.float64)
    CH = 2048
    for s in range(0, len(rows), CH):
        qq = q[rows[s:s + CH]].astype(np.float64)
        d = ((qq[:, None, :] - r64[None, :, :]) ** 2).sum(-1)
        out[s:s + CH] = d.min(1)
    return out


def _finish_job(md_strip, qs, rs, thr, dq):
    """md_strip [P, NBLK] device panel-mins for block-ordered queries; verify +
    repair. Returns md for block-ordered queries [N] (float64)."""
    md = np.maximum(md_strip.T.reshape(N).astype(np.float64), 0.0)
    bound = (thr[:, None] + dq).reshape(N)                # sound exclusion radius
    bad = np.flatnonzero(np.sqrt(md + E_DEV) > bound)
    _CACHE.setdefault("repairs", []).append(len(bad))
    if len(bad):
        md[bad] = _exact_rows(qs, rs, bad)
    return md


def kernel(xyz1: np.ndarray, xyz2: np.ndarray):
    xyz1 = np.asarray(xyz1, dtype=np.float32)
    xyz2 = np.asarray(xyz2, dtype=np.float32)
    B = xyz1.shape[0]
    assert xyz1.shape == (B, N, 3) and xyz2.shape == (B, N, 3)

    # 8 jobs: (batch, direction). direction 0: queries=xyz1 refs=xyz2 -> dist1
    jobs = []
    for b in range(B):
        jobs.append((xyz1[b], xyz2[b]))
        jobs.append((xyz2[b], xyz1[b]))

    in_maps = []
    host_state = []
    for (q, r) in jobs:
        oq = _kd_partition(q)
        qs = q[oq]
        idx, thr, dq = _build_panels(qs, r)
        panel = r[idx.reshape(-1)]                         # [NBLK*WE, 3]
        aT, _ = _lift(_query_feats(qs), _ref_feats(qs[:1]))
        _, gT = _lift(_query_feats(panel[:1]), _ref_feats(panel))
        in_maps.append({"aT": aT, "gT": gT})
        host_state.append((qs, r, oq, thr, dq))

    nc = _get_nc()
    _CACHE["last_in_maps"] = in_maps
    res = run_bass_kernel_spmd(nc, in_maps, core_ids=list(range(len(jobs))))
    _CACHE["last_results"] = res

    dist1 = np.empty((B, N), np.float32)
    dist2 = np.empty((B, N), np.float32)
    for j, (qs, r, oq, thr, dq) in enumerate(host_state):
        md_sorted = _finish_job(res.results[j]["md"], qs, r, thr, dq)
        md = np.empty(N, np.float64)
        md[oq] = md_sorted
        if j % 2 == 0:
            dist1[j // 2] = md.astype(np.float32)
        else:
            dist2[j // 2] = md.astype(np.float32)
    return dist1, dist2
